# revision 9
# baseline (speedup 1.0000x reference)
"""TransformerXL relative attention on 8 TRN2 NeuronCores (batch-parallel).

Per-core (one batch element):
  - load query^T / [mem|query]^T / pos_enc^T (f32), stream weights
  - projections via fp32r matmuls (1 cyc/row at N=512): qcbT/qpbT [hs,q],
    kT [hs,r], rT [hs,rel], v [r,hs]
  - position logits P[q,rel] per head -> bf16 -> DRAM scratch with padded
    rows (pad = -30000); read back with a skewed affine AP implementing
    TransformerXL rel_shift exactly (masked region reads pad -> exp -> 0)
  - scores psum = content matmul + identity-matmul(P_shifted);
    exp via ScalarE (scale=1/8) with accum_out giving softmax denominators
  - normalize attn (bf16), PE-transpose to attnT, PV matmuls (bf16,
    col-packed head pairs), output projection (fp32r), DMA out.
"""

import sys

if "/opt/trn_rl_repo" not in sys.path:
    sys.path.insert(0, "/opt/trn_rl_repo")

import numpy as np

B, Q, MEM, D, H, S = 8, 512, 512, 1024, 16, 64
R = Q + MEM  # 1024
L = 1536  # padded row pitch of the P scratch buffer (1024 data + 512 pad)
PAD_VAL = -30000.0
NKD = D // 128  # 8 contraction tiles
NI = D // 128  # 8 hs-tiles
NQT = Q // 128  # 4 q-tiles
NRT = R // 128  # 8 r-tiles

_CACHE = {}


def _build_nc():
    import concourse.bass as bass
    import concourse.mybir as mybir
    import concourse.tile as tile
    from concourse import bacc
    from concourse.bass import ds
    from concourse.masks import make_identity

    f32 = mybir.dt.float32
    bf16 = mybir.dt.bfloat16
    f32r = mybir.dt.float32r
    AF = mybir.ActivationFunctionType

    nc = bacc.Bacc("TRN2", target_bir_lowering=False)

    qTin = nc.dram_tensor("qT", [D, Q], f32r, kind="ExternalInput")
    refTin = nc.dram_tensor("refT", [D, R], f32r, kind="ExternalInput")
    posTin = nc.dram_tensor("posT", [D, R], f32r, kind="ExternalInput")
    Wq_d = nc.dram_tensor("Wq", [D, D], f32r, kind="ExternalInput")
    Wk_d = nc.dram_tensor("Wk", [D, D], f32r, kind="ExternalInput")
    Wv_d = nc.dram_tensor("Wv", [D, D], f32r, kind="ExternalInput")
    Wr_d = nc.dram_tensor("Wr", [D, D], f32r, kind="ExternalInput")
    Wo_d = nc.dram_tensor("Wo", [D, D], f32r, kind="ExternalInput")
    cb_d = nc.dram_tensor("cb", [128, NI], f32, kind="ExternalInput")
    pb_d = nc.dram_tensor("pb", [128, NI], f32, kind="ExternalInput")
    out_d = nc.dram_tensor("out", [Q, D], f32, kind="ExternalOutput")

    def r32(ap):
        return ap.bitcast(f32r)

    with tile.TileContext(nc) as tc:
        with (
            tc.tile_pool(name="persist", bufs=1) as persist,
            tc.tile_pool(name="dram", bufs=1, space="DRAM") as dram,
        ):
            ident = persist.tile([128, 128], bf16, tag="ident")
            make_identity(nc, ident)
            cb_sb = persist.tile([128, NI], f32, tag="cb")
            pb_sb = persist.tile([128, NI], f32, tag="pb")
            nc.sync.dma_start(out=cb_sb, in_=cb_d[:, :])
            nc.sync.dma_start(out=pb_sb, in_=pb_d[:, :])

            kT = persist.tile([128, NI, R], f32r, tag="kT")
            v_sb = persist.tile([128, NRT, D], bf16, tag="v")
            qcb = persist.tile([128, NI, Q], f32r, tag="qcb")
            qpb = persist.tile([128, NI, Q], f32r, tag="qpb")
            outT = persist.tile([128, NI, Q], f32r, tag="outT")

            Pdram = [
                dram.tile([Q * L], bf16, tag=f"pbuf{h}", name=f"pbuf{h}")
                for h in range(H)
            ]

            with tc.tile_pool(name="rtp", bufs=1) as rtp:
                rT = rtp.tile([128, NI, R], f32r, tag="rT")

                # ============ phase B: projections ============
                with (
                    tc.tile_pool(name="wst", bufs=3) as wst,
                    tc.tile_pool(name="pjB", bufs=8, space="PSUM") as pjB,
                ):
                    def proj(w_dram, rhs_sb_of, n_blocks, emit_out):
                        """generic: for each 512-col block nb, accumulate over kd:
                        psum[i] += w[kd][:, i*128:+128].T @ rhs(kd, nb); then emit."""
                        for nb in range(n_blocks):
                            psums = [
                                pjB.tile([128, 512], f32, tag="pj", name="pj")
                                for _ in range(NI)
                            ]
                            for kd in range(NKD):
                                wt = wst.tile([128, D], f32r, tag="w")
                                nc.gpsimd.dma_start(
                                    out=wt, in_=w_dram[kd * 128 : (kd + 1) * 128, :]
                                )
                                for i in range(NI):
                                    nc.tensor.matmul(
                                        psums[i],
                                        lhsT=r32(wt[:, i * 128 : (i + 1) * 128]),
                                        rhs=r32(rhs_sb_of(kd, nb)),
                                        start=(kd == 0),
                                        stop=(kd == NKD - 1),
                                    )
                            for i in range(NI):
                                emit_out(i, nb, psums[i])

                    # --- r projection (uses posT; free it after) ---
                    with tc.tile_pool(name="pos", bufs=1) as posp:
                        posT = posp.tile([128, NKD, R], f32r, tag="posT")
                        for kd in range(NKD):
                            nc.sync.dma_start(
                                out=posT[:, kd, :],
                                in_=posTin[kd * 128 : (kd + 1) * 128, :],
                            )

                        def emit_rT(i, nb, ps):
                            nc.scalar.copy(rT[:, i, ds(nb * 512, 512)], ps)

                        proj(Wr_d, lambda kd, nb: posT[:, kd, ds(nb * 512, 512)],
                             2, emit_rT)

                    # --- q projection ---
                    with tc.tile_pool(name="qin", bufs=1) as qinp:
                        qT_sb = qinp.tile([128, NKD, Q], f32r, tag="qTin")
                        for kd in range(NKD):
                            nc.sync.dma_start(
                                out=qT_sb[:, kd, :],
                                in_=qTin[kd * 128 : (kd + 1) * 128, :],
                            )

                        def emit_q(i, nb, ps):
                            nc.vector.tensor_scalar_add(
                                qcb[:, i, :], ps, cb_sb[:, i : i + 1]
                            )
                            nc.vector.tensor_scalar_add(
                                qpb[:, i, :], ps, pb_sb[:, i : i + 1]
                            )

                        proj(Wq_d, lambda kd, nb: qT_sb[:, kd, :], 1, emit_q)

                    # --- k and v projections (use refT) ---
                    with tc.tile_pool(name="refp", bufs=1) as refp:
                        refT = refp.tile([128, NKD, R], f32r, tag="refT")
                        for kd in range(NKD):
                            nc.sync.dma_start(
                                out=refT[:, kd, :],
                                in_=refTin[kd * 128 : (kd + 1) * 128, :],
                            )

                        def emit_kT(i, nb, ps):
                            nc.vector.tensor_copy(kT[:, i, ds(nb * 512, 512)], ps)

                        proj(Wk_d, lambda kd, nb: refT[:, kd, ds(nb * 512, 512)],
                             2, emit_kT)

                        # v natural [r, hs]: psum[rt] += refT[kd][:,rt].T @ Wv[kd][:, nb]
                        for nb in range(2):
                            psums = [
                                pjB.tile([128, 512], f32, tag="pj", name="pj")
                                for _ in range(NRT)
                            ]
                            for kd in range(NKD):
                                wt = wst.tile([128, D], f32r, tag="w")
                                nc.gpsimd.dma_start(
                                    out=wt,
                                    in_=Wv_d[kd * 128 : (kd + 1) * 128, :],
                                )
                                for rt in range(NRT):
                                    nc.tensor.matmul(
                                        psums[rt],
                                        lhsT=r32(
                                            refT[:, kd, ds(rt * 128, 128)]
                                        ),
                                        rhs=r32(wt[:, ds(nb * 512, 512)]),
                                        start=(kd == 0),
                                        stop=(kd == NKD - 1),
                                    )
                            for rt in range(NRT):
                                nc.vector.tensor_copy(
                                    v_sb[:, rt, ds(nb * 512, 512)], psums[rt]
                                )

                # ============ phases C/D: per-head ============
                with (
                    tc.tile_pool(name="pst", bufs=4) as pstp,
                    tc.tile_pool(name="psh", bufs=4) as pshp,
                    tc.tile_pool(name="attn", bufs=8) as attnp,
                    tc.tile_pool(name="attnT", bufs=2) as attnTp,
                    tc.tile_pool(name="den", bufs=8) as denp,
                    tc.tile_pool(name="ppsum", bufs=2, space="PSUM") as ppsum,
                    tc.tile_pool(name="scp", bufs=2, space="PSUM") as scp,
                    tc.tile_pool(name="trp", bufs=2, space="PSUM") as trpsum,
                    tc.tile_pool(name="pvp", bufs=1, space="PSUM") as pvp,
                ):
                    # ---- C(h): position logits -> bf16 -> DRAM (padded rows)
                    for h in range(H):
                        i_h, off = h // 2, (h % 2) * 64
                        for qt in range(NQT):
                            pst = pstp.tile([128, L], bf16, tag="pst")
                            nc.vector.memset(pst[:, R:L], PAD_VAL)
                            for rb in range(2):
                                pp = ppsum.tile([128, 512], f32, tag="pp")
                                nc.tensor.matmul(
                                    pp,
                                    lhsT=r32(
                                        qpb[off : off + 64, i_h, ds(qt * 128, 128)]
                                    ),
                                    rhs=r32(
                                        rT[off : off + 64, i_h, ds(rb * 512, 512)]
                                    ),
                                    start=True,
                                    stop=True,
                                )
                                nc.scalar.copy(pst[:, ds(rb * 512, 512)], pp)
                            import concourse.bass as bass_mod

                            wr_ap = bass_mod.AP(
                                tensor=Pdram[h].tensor,
                                offset=Pdram[h].offset + qt * 128 * L,
                                ap=[[L, 128], [1, L]],
                            )
                            nc.sync.dma_start(out=wr_ap, in_=pst)

                    # ---- D(h): scores, exp, normalize; T+PV per pair ----
                    import concourse.bass as bass_mod

                    attn_of = {}
                    for h in range(H):
                        i_h, off = h // 2, (h % 2) * 64
                        for qt in range(NQT):
                            attn = attnp.tile([128, R], bf16, tag="attn")
                            den = denp.tile([128, 4], f32, tag="den")
                            psh = pshp.tile([128, R], bf16, tag="psh")
                            rd_ap = bass_mod.AP(
                                tensor=Pdram[h].tensor,
                                offset=Pdram[h].offset + qt * 128 * (L - 1) + 511,
                                ap=[[L - 1, 128], [1, R]],
                            )
                            nc.sync.dma_start(out=psh, in_=rd_ap)
                            for rb in range(2):
                                sc = scp.tile([128, 512], f32, tag="sc")
                                nc.tensor.matmul(
                                    sc,
                                    lhsT=r32(
                                        qcb[off : off + 64, i_h, ds(qt * 128, 128)]
                                    ),
                                    rhs=r32(
                                        kT[off : off + 64, i_h, ds(rb * 512, 512)]
                                    ),
                                    start=True,
                                    stop=False,
                                )
                                nc.tensor.matmul(
                                    sc,
                                    lhsT=ident,
                                    rhs=psh[:, ds(rb * 512, 512)],
                                    start=False,
                                    stop=True,
                                    skip_group_check=True,
                                )
                                nc.scalar.activation(
                                    attn[:, ds(rb * 512, 512)],
                                    sc,
                                    AF.Exp,
                                    scale=0.125,
                                    accum_out=den[:, rb : rb + 1],
                                )
                            nc.vector.tensor_add(
                                den[:, 2:3], den[:, 0:1], den[:, 1:2]
                            )
                            nc.vector.reciprocal(den[:, 3:4], den[:, 2:3])
                            nc.vector.tensor_scalar_mul(attn, attn, den[:, 3:4])
                            attn_of[(h, qt)] = attn

                        if h % 2 == 1:
                            j = h // 2
                            h0, h1 = 2 * j, 2 * j + 1
                            aT = {}
                            for hh in (h0, h1):
                                attnT = attnTp.tile(
                                    [128, NRT, Q], bf16, tag="attnT"
                                )
                                for rt in range(NRT):
                                    trp = trpsum.tile([128, 512], bf16, tag="tr")
                                    for qt in range(NQT):
                                        nc.tensor.transpose(
                                            trp[:, ds(qt * 128, 128)],
                                            attn_of[(hh, qt)][
                                                :, ds(rt * 128, 128)
                                            ],
                                            ident,
                                        )
                                    if rt % 2 == 0:
                                        nc.vector.tensor_copy(
                                            attnT[:, rt, :], trp
                                        )
                                    else:
                                        nc.scalar.copy(attnT[:, rt, :], trp)
                                aT[hh] = attnT
                            pv = pvp.tile([128, 512], f32, tag="pv")
                            for hh in (h0, h1):
                                o2 = (hh % 2) * 64
                                for rt in range(NRT):
                                    nc.tensor.matmul(
                                        pv[o2 : o2 + 64, :],
                                        lhsT=v_sb[:, rt, ds(hh * 64, 64)],
                                        rhs=aT[hh][:, rt, :],
                                        start=(rt == 0),
                                        stop=(rt == NRT - 1),
                                        tile_position=(0, o2),
                                    )
                            nc.vector.tensor_copy(outT[:, j, :], pv)
                            for qt in range(NQT):
                                del attn_of[(h0, qt)]
                                del attn_of[(h1, qt)]

            # ============ output projection ============
            with (
                tc.tile_pool(name="wo", bufs=1) as wop,
                tc.tile_pool(name="ost", bufs=3) as ostp,
                tc.tile_pool(name="opj", bufs=2, space="PSUM") as opj,
            ):
                Wo_sb = wop.tile([128, NI, D], f32r, tag="Wo")
                for i in range(NI):
                    nc.sync.dma_start(
                        out=Wo_sb[:, i, :], in_=Wo_d[i * 128 : (i + 1) * 128, :]
                    )
                for qt in range(NQT):
                    for db in range(2):
                        op = opj.tile([128, 512], f32, tag="op")
                        for i in range(NI):
                            nc.tensor.matmul(
                                op,
                                lhsT=r32(outT[:, i, ds(qt * 128, 128)]),
                                rhs=r32(Wo_sb[:, i, ds(db * 512, 512)]),
                                start=(i == 0),
                                stop=(i == NI - 1),
                            )
                        ot = ostp.tile([128, 512], f32, tag="ot")
                        nc.vector.tensor_copy(ot, op)
                        nc.sync.dma_start(
                            out=out_d[
                                qt * 128 : (qt + 1) * 128, db * 512 : (db + 1) * 512
                            ],
                            in_=ot,
                        )

    return nc


def _get_nc():
    if "nc" not in _CACHE:
        nc = _build_nc()
        if not nc.is_finalized():
            nc.finalize()
        _CACHE["nc"] = nc
    return _CACHE["nc"]


def _prep_in_maps(inputs):
    q = np.asarray(inputs["query_seqs"], dtype=np.float32)
    mem = np.asarray(inputs["memory_seqs"], dtype=np.float32)
    pos = np.asarray(inputs["positional_encoding"], dtype=np.float32)
    Wq = np.ascontiguousarray(
        np.asarray(inputs["Wq"], dtype=np.float32).reshape(D, D)
    )
    Wk = np.ascontiguousarray(
        np.asarray(inputs["Wk"], dtype=np.float32).reshape(D, D)
    )
    Wv = np.ascontiguousarray(
        np.asarray(inputs["Wv"], dtype=np.float32).reshape(D, D)
    )
    Wr = np.ascontiguousarray(
        np.asarray(inputs["Wr"], dtype=np.float32).reshape(D, D)
    )
    Wo = np.ascontiguousarray(
        np.asarray(inputs["Wo"], dtype=np.float32).reshape(D, D)
    )
    cb = np.ascontiguousarray(
        np.asarray(inputs["content_bias"], dtype=np.float32)
        .reshape(D)
        .reshape(NI, 128)
        .T
    )
    pb = np.ascontiguousarray(
        np.asarray(inputs["position_bias"], dtype=np.float32)
        .reshape(D)
        .reshape(NI, 128)
        .T
    )
    posT = np.ascontiguousarray(pos.T)

    in_maps = []
    for b in range(B):
        refT = np.ascontiguousarray(
            np.concatenate([mem[b], q[b]], axis=0).T
        )
        qT = np.ascontiguousarray(q[b].T)
        in_maps.append(
            dict(
                qT=qT, refT=refT, posT=posT,
                Wq=Wq, Wk=Wk, Wv=Wv, Wr=Wr, Wo=Wo, cb=cb, pb=pb,
            )
        )
    return in_maps


def run_spmd(inputs, **kwargs):
    """Run on 8 cores; returns (output [B,Q,D], BassKernelResults)."""
    from concourse.bass_utils import run_bass_kernel_spmd

    nc = _get_nc()
    in_maps = _prep_in_maps(inputs)
    res = run_bass_kernel_spmd(nc, in_maps, core_ids=list(range(B)), **kwargs)
    out = np.stack([r["out"] for r in res.results], axis=0).astype(np.float32)
    return out, res


def kernel(**inputs) -> np.ndarray:
    out, _ = run_spmd(inputs)
    return out
